# revision 27
# baseline (speedup 1.0000x reference)
"""Trainium2 Bass kernel for a 2-layer GAT (GATConv x2 + linear head).

Strategy (8 NeuronCores, dst-node sharded, zero cross-core reduction):
  - Nodes are snake-dealt by in-degree to 32 (core, bucket) chunks of 3125
    real nodes (+11 pad) -> each core owns 12544 local rows = 98 blocks of
    128; bucket j = local rows [j*3136, (j+1)*3136) of every core.
  - Table row order is bucket-major ("bidx"): bucket j holds the 8 cores'
    chunk-j rows contiguously (25088 rows), so gather indices are int16.
  - Edges are grouped per (src-bucket, dst-block run); within each (b, block)
    dst nodes are ordered by bucket-b in-degree so the slot rectangle
    [128 nodes x k slots] is near-tight.
  - Gather: custom SWDGE dma_gather, 1024-idx instructions (ucode ring cap),
    round-robined over 4 queues. Layer-1 rows are 16B f32 (x + a_s);
    layer-2 rows are 132B f32 (h2 + a_s2), both on 256B stride. (bf16 rows
    were tried and are ~2x SLOWER per descriptor on HW: unaligned 66B elems.)
  - Per-edge softmax: z = a_s[src] + a_d[dst], Lrelu+Exp on ACT, weight &
    segment-sum via in-place multiply + strided tensor_reduce on DVE.
    Segment-max subtraction is skipped (logits are in [-5, 5]; softmax is
    shift-invariant).
  - Partial sums accumulate in an SBUF slab across up to 8 dst blocks and
    flush with one 1024-row dma_scatter_add (CCE f32) per batch, emitted a
    batch late so the Pool engine preps gathers instead of stalling.
  - W1 is applied at finalize on DVE (3 broadcast FMAs); W2 via PE
    transpose + block-diag matmul.
  - Layer-2 node table is exchanged with ONE packed 132B-row AllGather
    (1.65MB/core), then restrided locally to 256B-row bucket tables.

kernel(**inputs) -> np.ndarray [100000, 1] float32.
"""

import numpy as np

import concourse.bass as bass
import concourse.mybir as mybir
import concourse.tile as tile
from concourse import bacc, ap_utils
from concourse._compat import exact_div
from concourse.bass_utils import run_bass_kernel_spmd

# ---------------------------------------------------------------- constants
N = 100000
E = 3200000
NC = 8
P = 128
NPC = 12544                # local rows per core (4 chunks of 3136)
NBLK = NPC // P            # 98
CHK = 3136                 # local rows per bucket-chunk
REAL = 3125                # real nodes per (core, bucket) chunk
NB = 4                     # src buckets
BUCKET = NC * CHK          # 25088 table rows per bucket
NRANK = NC * NPC           # 100352
ROWF = 64                  # f32 table row stride (256B)
ROW2 = 128                 # bf16 table row stride (256B)
L1W = 4                    # layer-1 gather width: x(3) + a_s1 (f32)
L2W = 33                   # layer-2 gather width: h2(32) + a_s2 (bf16)
NEG = 0.2
A_S_PAD = -1.0e9
EPS = 1e-16
CAP = 1024                 # max idxs per SWDGE gather/scatter (ucode ring cap)
SMAX = 192                 # max slots (per partition) per compute group
MMAX = 8                   # max blocks per group
MBATCH = 8                 # scatter slab width (blocks; 8*128 = 1024 idxs)
GSLAB = 4096               # gidx slab columns (int16) per load
DT = mybir.dt.float32
DT2 = mybir.dt.float32
DI = mybir.dt.int16
DUMMY = CHK - 1            # pad row inside every bucket (core 0 chunk tail)

# consts column layout
C_W1 = 0        # [12, 128] block-diag W1 (parts 0:12, unused cols ok)
C_W2 = 128      # [128, 128] block-diag W2
C_VS2 = 256     # 32
C_VD2 = 288     # 32
C_WL = 320      # 32
C_ID = 352      # 128 identity
C_LM = 480      # 98  local pad mask (0 / -1e9)
C_XM = 578      # 784 bidx pad mask (0 / -1e9)
C_W1R = 1362    # 3 x 32 rows of W1, replicated across partitions
C_END = 1472


# ------------------------------------------------------- raw SWDGE ops
def dma_gather_raw(gp, out_ap, in_ap, idxs_ap, num_idxs, elem_size, elem_step,
                   queue_num=0):
    assert idxs_ap.dtype == DI
    assert in_ap.dtype == out_ap.dtype
    assert in_ap.space == bass.MemorySpace.DRAM
    assert ap_utils.ap_is_contiguous(out_ap.ap[1:])
    assert ap_utils.ap_is_contiguous(idxs_ap.ap[1:])
    assert in_ap.ap[-1][1] == out_ap.ap[-1][1] == elem_size
    assert out_ap.ap[0][1] * out_ap.ap[1][1] >= num_idxs
    assert in_ap.ap[0][0] == elem_step
    stride_bytes_256 = exact_div(elem_step * mybir.dt.size(in_ap.dtype), 256)
    assert 0 < stride_bytes_256 < 256
    _in_ap = gp.lower_ap_dma(in_ap, for_custom_bir_dma=True)
    _idxs_ap = gp.lower_ap(idxs_ap)
    _out_ap = gp.lower_ap(out_ap)
    return gp.add_instruction(
        mybir.InstDMAGatherAnt(
            name=gp.bass.get_next_instruction_name(),
            ins=[*_in_ap, _idxs_ap, gp.lower_val_access(gp.to_reg(num_idxs))],
            outs=[_out_ap],
            transpose=False,
            num_idxs=num_idxs,
            elem_size=elem_size,
            stride_bytes_256=stride_bytes_256,
            gen_mode=0,
            single_packet=True,
            queue_num=queue_num,
            sbuf_tokens_per_rank=0,
            sbuf_free_dim_per_rank=0,
            sbuf_free_dim_pad_per_rank=0,
            sbuf_byte_offset=0,
        ))


def dma_scatter_add_raw(gp, out_ap, in_ap, idxs_ap, num_idxs, elem_size,
                        elem_step, queue_num=0):
    assert idxs_ap.dtype == DI
    assert in_ap.dtype == out_ap.dtype
    assert in_ap.space == bass.MemorySpace.SBUF
    assert out_ap.space == bass.MemorySpace.DRAM
    assert ap_utils.ap_is_contiguous(in_ap.ap[1:])
    assert ap_utils.ap_is_contiguous(idxs_ap.ap[1:])
    assert in_ap.ap[0][1] * in_ap.ap[1][1] >= num_idxs
    assert in_ap.ap[-1][1] == out_ap.ap[-1][1] == elem_size
    assert out_ap.ap[0][0] == elem_step
    stride_bytes_256 = exact_div(elem_step * mybir.dt.size(out_ap.dtype), 256)
    assert stride_bytes_256 < 256
    _in_ap = gp.lower_ap(in_ap)
    _idxs_ap = gp.lower_ap(idxs_ap)
    return gp.add_instruction(
        mybir.InstDMAScatterAddAnt(
            name=gp.bass.get_next_instruction_name(),
            ins=[_in_ap, _idxs_ap, gp.lower_val_access(gp.to_reg(num_idxs))],
            outs=[*gp.lower_ap_dma(out_ap, for_custom_bir_dma=True)],
            num_idxs=num_idxs,
            elem_size=elem_size,
            stride_bytes_256=stride_bytes_256,
            read_from_swizzled=False,
            gen_mode=0,
            single_packet=True,
            queue_num=queue_num,
            sbuf_tokens_per_rank=0,
        ))


def wrap16(idx):
    """[n] int -> SWDGE wrapped idx layout [128, n/16] int16 (8x replicated)."""
    n = len(idx)
    n16 = ((n + 15) // 16) * 16
    buf = np.full(n16, -1, np.int16)
    buf[:n] = idx
    w = buf.reshape(n16 // 16, 16).T
    return np.tile(w, (8, 1))


def rank_to_bidx(rank):
    c, l = rank // NPC, rank % NPC
    b, i = l // CHK, l % CHK
    return b * BUCKET + c * CHK + i


# ------------------------------------------------------- host preprocessing
def preprocess(edge_index):
    src = np.concatenate([edge_index[0].astype(np.int64),
                          np.arange(N, dtype=np.int64)])
    dst = np.concatenate([edge_index[1].astype(np.int64),
                          np.arange(N, dtype=np.int64)])

    deg = np.bincount(dst, minlength=N)
    order = np.argsort(-deg, kind="stable")
    pos = np.arange(N)
    rnd, lane = pos // 32, pos % 32
    c32 = np.where(rnd % 2 == 0, lane, 31 - lane)
    core, buck = c32 % NC, c32 // NC
    node2rank = np.empty(N, np.int64)
    node2rank[order] = core * NPC + buck * CHK + rnd  # rnd < 3125

    srank = node2rank[src]
    drank = node2rank[dst]
    dcore = drank // NPC

    per_core = []
    counts = np.zeros((NC, NB, NPC), np.int64)
    for c in range(NC):
        m = dcore == c
        s_c, d_c = srank[m], drank[m] % NPC
        sl = s_c % NPC
        b_c = sl // CHK
        inb = (s_c // NPC) * CHK + sl - b_c * CHK  # in-bucket idx < 25088
        per_core.append((inb, d_c, b_c))
        for b in range(NB):
            mm = b_c == b
            counts[c, b] = np.bincount(d_c[mm], minlength=NPC)

    perms = np.empty((NC, NB, NPC), np.int64)
    for c in range(NC):
        for b in range(NB):
            perms[c, b] = np.argsort(-counts[c, b], kind="stable")

    # unified k per (bucket, block) across cores
    kk = np.zeros((NB, NBLK), np.int64)
    for b in range(NB):
        cnt = np.take_along_axis(counts[:, b], perms[:, b], axis=1)
        kk[b] = cnt.reshape(NC, NBLK, P).max(axis=(0, 2))

    # compute groups: consecutive same-k blocks, M <= MMAX, M*k <= SMAX
    groups = []  # (b, g0, m, k)
    for b in range(NB):
        g = 0
        while g < NBLK:
            k = int(kk[b, g])
            if k == 0:
                g += 1
                continue
            mlim = max(1, min(MMAX, SMAX // k))
            m = 1
            while (m < mlim and g + m < NBLK and kk[b, g + m] == k):
                m += 1
            groups.append((b, g, m, k))
            g += m

    # per-core gather index streams (identical program, different data)
    gidx_cols = []   # per core: per group flat rect idx arrays
    sidx_cols = []   # per core: per group node lists (local ids)
    adidx_cols = []
    for c in range(NC):
        inb_c, d_c, b_c = per_core[c]
        gparts, sparts = [], []
        for b in range(NB):
            mm = b_c == b
            sb, db = inb_c[mm], d_c[mm]
            o = np.argsort(db, kind="stable")
            sb, db = sb[o], db[o]
            starts = np.searchsorted(db, np.arange(NPC))
            ends = np.searchsorted(db, np.arange(NPC) + 1)
            for (bb, g0, m, k) in groups:
                if bb != b:
                    continue
                nodes = perms[c, b, g0 * P:(g0 + m) * P]
                rect = np.full((m * k, P), DUMMY, np.int64)
                for u in range(m):
                    nd = nodes[u * P:(u + 1) * P]
                    for p, nloc in enumerate(nd):
                        s0, s1 = starts[nloc], ends[nloc]
                        rect[u * k:u * k + (s1 - s0), p] = sb[s0:s1]
                gparts.append(rect.reshape(-1))
                sparts.append(nodes)
        gidx_cols.append(gparts)
        sidx_cols.append(sparts)
        adidx_cols.append([perms[c, b] for b in range(NB)])

    # gather stream: per group, pieces of <= CAP idxs
    gstream = [[] for _ in range(NC)]
    meta_g = []      # per group: list of (piece_col0, piece_cols, tp, t0)
    col = 0
    for gi, (b, g0, m, k) in enumerate(groups):
        S = m * k
        pieces = []
        t0 = 0
        while t0 < S:
            tp = min(CAP // P, S - t0)
            pieces.append((col, tp * 8, tp, t0))
            for c in range(NC):
                part = gidx_cols[c][gi][t0 * P:(t0 + tp) * P]
                gstream[c].append(wrap16(part))
            col += tp * 8
            t0 += tp
        meta_g.append(pieces)
    gidx_arr = [np.concatenate(gstream[c], axis=1) if gstream[c]
                else np.zeros((P, 8), np.int16) for c in range(NC)]

    # scatter stream: slab batches of up to MBATCH blocks across groups
    sstream = [[] for _ in range(NC)]
    meta_s = []       # per group: (batch_id, slab_off_m)
    batches = []      # per batch: (scol, cols, n_rows)
    scol = 0
    cur_off = 0
    cur_scol = scol
    for gi, (b, g0, m, k) in enumerate(groups):
        if cur_off + m > MBATCH:
            batches.append((cur_scol, (scol - cur_scol) // 1, cur_off * P))
            cur_off = 0
            cur_scol = scol
        meta_s.append((len(batches), cur_off))
        for c in range(NC):
            sstream[c].append(wrap16(sidx_cols[c][gi]))
        scol += m * 8
        cur_off += m
    if cur_off:
        batches.append((cur_scol, scol - cur_scol, cur_off * P))
    sidx_arr = [np.concatenate(sstream[c], axis=1) for c in range(NC)]

    # a_d idx stream: per bucket, pieces of CAP
    adstream = [[] for _ in range(NC)]
    meta_ad = []
    acol = 0
    for b in range(NB):
        pieces = []
        t0 = 0
        while t0 < NBLK:
            tp = min(CAP // P, NBLK - t0)
            pieces.append((acol, tp * 8, tp, t0))
            for c in range(NC):
                part = adidx_cols[c][b][t0 * P:(t0 + tp) * P]
                adstream[c].append(wrap16(part))
            acol += tp * 8
            t0 += tp
        meta_ad.append(pieces)
    adidx_arr = [np.concatenate(adstream[c], axis=1) for c in range(NC)]

    return dict(node2rank=node2rank, groups=groups, meta_g=meta_g,
                meta_s=meta_s, batches=batches, meta_ad=meta_ad,
                gidx=gidx_arr, sidx=sidx_arr, adidx=adidx_arr, perms=perms,
                gcols=col, scols=scol, adcols=acol)


# ------------------------------------------------------- program builder
def build_program(prep, weights, replicate=1):
    groups = prep["groups"]
    meta_g, meta_s, meta_ad = prep["meta_g"], prep["meta_s"], prep["meta_ad"]
    batches = prep["batches"]
    W1 = weights["W1"]; W2 = weights["W2"]
    vs1 = W1 @ weights["att_src1"]   # [3]
    vd1 = W1 @ weights["att_dst1"]
    b1 = weights["b1"]; b2 = weights["b2"]
    bl = float(weights["bl"][0])
    if np.abs(b1).max() > 0 or np.abs(b2).max() > 0:
        raise NotImplementedError("nonzero b1/b2")

    nc = bacc.Bacc("TRN2", target_bir_lowering=False, debug=False,
                   enable_asserts=False, num_devices=NC,
                   num_swdge_queues=4)

    # ---- external tensors
    xfull = nc.dram_tensor("xfull", [P, NC * NBLK, 3], DT, kind="ExternalInput")
    xb = nc.dram_tensor("xb", [P, NB, NBLK, 3], DT, kind="ExternalInput")
    gidx_d = nc.dram_tensor("gidx", [P, prep["gcols"]], DI, kind="ExternalInput")
    sidx_d = nc.dram_tensor("sidx", [P, prep["scols"]], DI, kind="ExternalInput")
    adidx_d = nc.dram_tensor("adidx", [P, prep["adcols"]], DI, kind="ExternalInput")
    consts = nc.dram_tensor("consts", [P, C_END], DT, kind="ExternalInput")
    y_d = nc.dram_tensor("y", [NPC, 1], DT, kind="ExternalOutput")

    # ---- internal DRAM
    tab1 = nc.dram_tensor("tab1", [NB * BUCKET, ROWF], DT)
    agin2 = nc.dram_tensor("agin2", [NPC, L2W], DT2)       # packed 66B rows
    tab2p = nc.dram_tensor("tab2p", [NC * NPC, L2W], DT2, addr_space="Shared")
    tab2c = [nc.dram_tensor(f"tab2c{j}", [BUCKET, ROWF], DT2)
             for j in range(NB)]
    part1 = nc.dram_tensor("part1", [NPC, ROWF], DT)
    part2 = nc.dram_tensor("part2", [NPC, ROWF], DT)

    with tile.TileContext(nc) as tc:
        with tc.tile_pool(name="const", bufs=1) as cpool, \
             tc.tile_pool(name="chunk", bufs=3) as chpool, \
             tc.tile_pool(name="small", bufs=3) as zpool, \
             tc.tile_pool(name="slab", bufs=3) as slpool, \
             tc.tile_pool(name="gix", bufs=2) as gixpool, \
             tc.tile_pool(name="psum", bufs=2, space="PSUM") as pspool:

            ct = cpool.tile([P, C_END], DT)
            nc.sync.dma_start(ct[:], consts[:])
            W1diag = ct[:, C_W1:C_W1 + 128]      # valid on partitions 0:12
            W2diag = ct[:, C_W2:C_W2 + 128]
            vs2bc = ct[:, C_VS2:C_VS2 + 32]
            vd2bc = ct[:, C_VD2:C_VD2 + 32]
            Wlbc = ct[:, C_WL:C_WL + 32]
            ident = ct[:, C_ID:C_ID + 128]
            lmask = ct[:, C_LM:C_LM + NBLK]
            xmask = ct[:, C_XM:C_XM + NC * NBLK]

            sixt = cpool.tile([P, prep["scols"]], DI, tag="sixt")
            nc.sync.dma_start(sixt[:], sidx_d[:])
            adixt = cpool.tile([P, prep["adcols"]], DI, tag="adixt")
            nc.sync.dma_start(adixt[:], adidx_d[:])
            adcol1 = cpool.tile([P, NB, NBLK], DT, tag="adcol1")
            adcol2 = cpool.tile([P, NB, NBLK], DT, tag="adcol2")

            qrr = [0]

            def nextq():
                qrr[0] = (qrr[0] + 1) % 4
                return qrr[0]

            def body():
                # zero-init DRAM accumulators
                zpool2 = tc.tile_pool(name="zz", bufs=1)
                spool = zpool2.__enter__()
                QZ = NBLK * ROWF // 2
                zt = spool.tile([P, QZ], DT, tag="zt")
                nc.vector.memset(zt[:], 0.0)
                for arr in (part1, part2):
                    zap = arr[:].rearrange("(q a b) c -> q a (b c)", a=P, q=2)
                    for q in range(2):
                        nc.sync.dma_start(zap[q], zt[:])

                # ---------------- stage 1: build tab1 (bidx order, all ranks)
                GH = NC * NBLK // 2        # half the bidx column space
                tab1v = tab1[:, 0:L1W].rearrange("(h g p) w -> h p g w",
                                                 p=P, h=2)
                for h in range(2):
                    xf = spool.tile([P, GH, 3], DT, tag="xf")
                    nc.sync.dma_start(xf[:], xfull[:, h * GH:(h + 1) * GH, :])
                    st1 = spool.tile([P, GH, L1W], DT, tag="st1")
                    nc.vector.tensor_copy(out=st1[:, :, 0:3], in_=xf[:])
                    tmpA = spool.tile([P, GH], DT, tag="tmpA")
                    nc.vector.tensor_scalar_mul(st1[:, :, 3], xf[:, :, 0],
                                                float(vs1[0]))
                    nc.vector.tensor_scalar_mul(tmpA[:], xf[:, :, 1],
                                                float(vs1[1]))
                    nc.vector.tensor_tensor(out=st1[:, :, 3], in0=st1[:, :, 3],
                                            in1=tmpA[:], op=mybir.AluOpType.add)
                    nc.vector.tensor_scalar_mul(tmpA[:], xf[:, :, 2],
                                                float(vs1[2]))
                    nc.vector.tensor_tensor(out=st1[:, :, 3], in0=st1[:, :, 3],
                                            in1=tmpA[:], op=mybir.AluOpType.add)
                    # pad rows (x host-zeroed): a_s += -1e9 mask (bidx order)
                    nc.vector.tensor_tensor(out=st1[:, :, 3], in0=st1[:, :, 3],
                                            in1=xmask[:, h * GH:(h + 1) * GH],
                                            op=mybir.AluOpType.add)
                    nc.sync.dma_start(tab1v[h], st1[:])

                # a_d1 per bucket from xb (already permuted by host)
                xbt = spool.tile([P, NB, NBLK, 3], DT, tag="xbt")
                nc.sync.dma_start(xbt[:], xb[:])
                tmpB = spool.tile([P, NB, NBLK], DT, tag="tmpB")
                nc.vector.tensor_scalar_mul(adcol1[:], xbt[:, :, :, 0], float(vd1[0]))
                nc.vector.tensor_scalar_mul(tmpB[:], xbt[:, :, :, 1], float(vd1[1]))
                nc.vector.tensor_tensor(out=adcol1[:], in0=adcol1[:], in1=tmpB[:],
                                        op=mybir.AluOpType.add)
                nc.vector.tensor_scalar_mul(tmpB[:], xbt[:, :, :, 2], float(vd1[2]))
                nc.vector.tensor_tensor(out=adcol1[:], in0=adcol1[:], in1=tmpB[:],
                                        op=mybir.AluOpType.add)
                zpool2.__exit__(None, None, None)

                # ---------------- edge phase (shared for both layers)
                def edge_phase(tab_of_b, part, W, dtype, adcol):
                    """tab_of_b(b) -> DRAM AP of bucket b's table [BUCKET, W]
                    (row stride 256B); dtype = table dtype."""
                    asoff = W - 1
                    step = ROWF if dtype == DT else ROW2
                    psl = {"tile": None, "bid": -1}
                    pend = []          # delayed scatter flushes

                    # precompute gidx slab bases (pieces consume cols in order)
                    slab_bases = [0]
                    for pieces in meta_g:
                        for (col0, cols, tp, t0) in pieces:
                            if col0 + cols > slab_bases[-1] + GSLAB:
                                slab_bases.append(col0)
                    slab_tiles = {}

                    def load_slab(si):
                        if si >= len(slab_bases) or si in slab_tiles:
                            return
                        t = gixpool.tile([P, GSLAB], DI, tag="gslab")
                        base = slab_bases[si]
                        csz = min(GSLAB, prep["gcols"] - base)
                        nc.sync.dma_start(t[:, 0:csz],
                                          gidx_d[:, base:base + csz])
                        slab_tiles[si] = t

                    load_slab(0)
                    load_slab(1)
                    cur_si = [0]

                    def gix(col0, cols):
                        si = cur_si[0]
                        if col0 + cols > slab_bases[si] + GSLAB:
                            si += 1
                            cur_si[0] = si
                            load_slab(si + 1)   # prefetch one ahead
                        b0 = col0 - slab_bases[si]
                        return slab_tiles[si][:, b0:b0 + cols]

                    def flush(bid, tile_):
                        (bcol, bcols, brows) = batches[bid]
                        dma_scatter_add_raw(
                            nc.gpsimd, part[:, 0:W], tile_[:],
                            sixt[:, bcol:bcol + bcols], brows, W, ROWF,
                            queue_num=nextq())

                    for gi, (b, g0, m, k) in enumerate(groups):
                        S = m * k
                        chunk = chpool.tile([P, SMAX, W], dtype, tag=f"ch{W}")
                        for (col0, cols, tp, t0) in meta_g[gi]:
                            dma_gather_raw(
                                nc.gpsimd, chunk[:, t0:t0 + tp, :],
                                tab_of_b(b), gix(col0, cols), tp * P, W, step,
                                queue_num=nextq())
                        # emit the previous batch's scatter only now, so the
                        # Pool engine preps this group's gathers first instead
                        # of stalling on the slab data dependency
                        while pend and pend[0][0] < meta_s[gi][0]:
                            flush(*pend.pop(0))
                        # z = a_s + a_d ; lrelu ; exp
                        z = zpool.tile([P, SMAX], DT, tag="z")
                        if dtype == DT:
                            nc.vector.tensor_tensor(
                                out=z[:, 0:S].rearrange("p (m k) -> p m k", m=m),
                                in0=chunk[:, 0:S, asoff].rearrange(
                                    "p (m k) -> p m k", m=m),
                                in1=adcol[:, b, g0:g0 + m].rearrange(
                                    "p (m o) -> p m o", o=1).to_broadcast(
                                    [P, m, k]),
                                op=mybir.AluOpType.add)
                        else:
                            zc = zpool.tile([P, SMAX], DT, tag="zc")
                            nc.vector.tensor_copy(out=zc[:, 0:S],
                                                  in_=chunk[:, 0:S, asoff])
                            nc.vector.tensor_tensor(
                                out=z[:, 0:S].rearrange("p (m k) -> p m k", m=m),
                                in0=zc[:, 0:S].rearrange("p (m k) -> p m k", m=m),
                                in1=adcol[:, b, g0:g0 + m].rearrange(
                                    "p (m o) -> p m o", o=1).to_broadcast(
                                    [P, m, k]),
                                op=mybir.AluOpType.add)
                        z2 = zpool.tile([P, SMAX], DT, tag="z2")
                        nc.scalar.activation(z2[:, 0:S], z[:, 0:S],
                                             mybir.ActivationFunctionType.Copy,
                                             scale=NEG)
                        nc.vector.tensor_tensor(out=z[:, 0:S], in0=z[:, 0:S],
                                                in1=z2[:, 0:S],
                                                op=mybir.AluOpType.max)
                        ex = zpool.tile([P, SMAX], dtype, tag=f"ex{W}")
                        nc.scalar.activation(ex[:, 0:S], z[:, 0:S],
                                             mybir.ActivationFunctionType.Exp)
                        # in-place weight multiply on value columns
                        nc.vector.tensor_tensor(
                            out=chunk[:, 0:S, 0:W - 1],
                            in0=chunk[:, 0:S, 0:W - 1],
                            in1=ex[:, 0:S].to_broadcast([P, S, W - 1]),
                            op=mybir.AluOpType.mult)
                        (bid, off) = meta_s[gi]
                        if psl["bid"] != bid or psl["tile"] is None:
                            psl["tile"] = slpool.tile([P, MBATCH, W], DT,
                                                      name=f"psl{W}",
                                                      tag=f"sl{W}")
                            psl["bid"] = bid
                        partial = psl["tile"][:, off:off + m, :]
                        nc.vector.tensor_reduce(
                            out=partial[:, :, 0:W - 1],
                            in_=chunk[:, 0:S, :].rearrange(
                                "p (m k) w -> p m w k", m=m)[:, :, 0:W - 1, :],
                            axis=mybir.AxisListType.X, op=mybir.AluOpType.add)
                        nc.vector.tensor_reduce(
                            out=partial[:, :, W - 1],
                            in_=ex[:, 0:S].rearrange("p (m k) -> p m k", m=m),
                            axis=mybir.AxisListType.X, op=mybir.AluOpType.add)
                        if gi + 1 >= len(groups) or meta_s[gi + 1][0] != bid:
                            pend.append((bid, psl["tile"]))
                            psl["tile"] = None
                    for args in pend:
                        flush(*args)

                edge_phase(lambda b: tab1[BUCKET * b:BUCKET * (b + 1), 0:L1W],
                           part1, L1W, DT, adcol1)

                # ---------------- finalize 1 -> agin2 (packed bf16), AllGather
                f1pool = tc.tile_pool(name="f1", bufs=1)
                spool = f1pool.__enter__()
                pt1 = spool.tile([P, NBLK, L1W], DT, tag="pt1")
                nc.sync.dma_start(
                    pt1[:], part1[:, 0:L1W].rearrange("(g p) w -> p g w", p=P))
                rec1 = spool.tile([P, NBLK], DT, tag="rec1")
                nc.vector.tensor_scalar_add(rec1[:], pt1[:, :, L1W - 1], EPS)
                nc.vector.reciprocal(rec1[:], rec1[:])
                vst1 = spool.tile([P, NBLK, 3], DT, tag="vst1")
                nc.vector.tensor_tensor(out=vst1[:], in0=pt1[:, :, 0:3],
                                        in1=rec1[:].to_broadcast([P, NBLK, 3]),
                                        op=mybir.AluOpType.mult)
                # h1 = relu(vst1 @ W1) on DVE: 3 broadcast FMAs (no PE chain)
                hfull = spool.tile([P, NBLK, 32], DT, tag="hfull")
                tmpw = spool.tile([P, 16, 32], DT, tag="tmpw")
                for u in range(0, NBLK, 16):
                    nu = min(16, NBLK - u)
                    for w in range(3):
                        w1r = ct[:, C_W1R + 32 * w:C_W1R + 32 * (w + 1)]
                        dst = (hfull[:, u:u + nu, :] if w == 0
                               else tmpw[:, 0:nu, :])
                        nc.vector.tensor_tensor(
                            out=dst,
                            in0=vst1[:, u:u + nu, w].rearrange(
                                "p (g o) -> p g o", o=1).to_broadcast(
                                [P, nu, 32]),
                            in1=w1r.rearrange(
                                "p (o w) -> p o w", o=1).to_broadcast(
                                [P, nu, 32]),
                            op=mybir.AluOpType.mult)
                        if w:
                            nc.vector.tensor_tensor(
                                out=hfull[:, u:u + nu, :],
                                in0=hfull[:, u:u + nu, :],
                                in1=tmpw[:, 0:nu, :],
                                op=mybir.AluOpType.add)
                nc.scalar.activation(hfull[:], hfull[:],
                                     mybir.ActivationFunctionType.Relu)
                # stage rows: [h2 (bf16 x32), a_s2] (a_d2 stays local)
                st2 = spool.tile([P, NBLK, L2W], DT2, tag="st2")
                nc.vector.tensor_copy(out=st2[:, :, 0:32], in_=hfull[:])
                tmp2 = spool.tile([P, 8, 32], DT, tag="tmp2")
                col2 = spool.tile([P, NBLK], DT, tag="col2")
                for u in range(0, NBLK, 8):
                    nu = min(8, NBLK - u)
                    nc.vector.tensor_tensor(
                        out=tmp2[:, 0:nu, :], in0=hfull[:, u:u + nu, :],
                        in1=vs2bc.rearrange("p (o w) -> p o w", o=1).to_broadcast([P, nu, 32]),
                        op=mybir.AluOpType.mult)
                    nc.vector.tensor_reduce(out=col2[:, u:u + nu],
                                            in_=tmp2[:, 0:nu, :],
                                            axis=mybir.AxisListType.X,
                                            op=mybir.AluOpType.add)
                nc.vector.tensor_tensor(out=col2[:], in0=col2[:],
                                        in1=lmask, op=mybir.AluOpType.add)
                nc.vector.tensor_copy(out=st2[:, :, 32], in_=col2[:])
                nc.sync.dma_start(
                    agin2[:].rearrange("(g p) w -> p g w", p=P), st2[:])
                nc.gpsimd.collective_compute(
                    "AllGather", mybir.AluOpType.bypass,
                    replica_groups=[list(range(NC))],
                    ins=[agin2[:]], outs=[tab2p[:]])
                # restride: core-major packed rows -> bucket tables (256B rows)
                for j in range(NB):
                    for c in range(NC):
                        nc.sync.dma_start(
                            tab2c[j][c * CHK:(c + 1) * CHK, 0:L2W],
                            tab2p[c * NPC + j * CHK:c * NPC + (j + 1) * CHK, :])

                # a_d2: write to part1 col 0 (dead), gather back in perm order
                for u in range(0, NBLK, 8):
                    nu = min(8, NBLK - u)
                    nc.vector.tensor_tensor(
                        out=tmp2[:, 0:nu, :], in0=hfull[:, u:u + nu, :],
                        in1=vd2bc.rearrange("p (o w) -> p o w", o=1).to_broadcast([P, nu, 32]),
                        op=mybir.AluOpType.mult)
                    nc.vector.tensor_reduce(out=col2[:, u:u + nu],
                                            in_=tmp2[:, 0:nu, :],
                                            axis=mybir.AxisListType.X,
                                            op=mybir.AluOpType.add)
                nc.sync.dma_start(
                    part1[:, 0:1].rearrange("(g p) w -> p (g w)", p=P),
                    col2[:])
                for b in range(NB):
                    for (col0, cols, tp, t0) in meta_ad[b]:
                        dma_gather_raw(
                            nc.gpsimd,
                            adcol2[:, b, t0:t0 + tp].rearrange(
                                "p (g o) -> p g o", o=1),
                            part1[:, 0:1], adixt[:, col0:col0 + cols],
                            tp * P, 1, ROWF, queue_num=nextq())
                f1pool.__exit__(None, None, None)

                # ---------------- layer 2 edge phase (bf16 tables)
                edge_phase(lambda b: tab2c[b][:, 0:L2W],
                           part2, L2W, DT2, adcol2)

                # ---------------- finalize 2 -> y
                f2pool = tc.tile_pool(name="f2", bufs=1)
                spool = f2pool.__enter__()
                pt2 = spool.tile([P, NBLK, L2W], DT, tag="pt2")
                nc.sync.dma_start(
                    pt2[:], part2[:, 0:L2W].rearrange("(g p) w -> p g w", p=P))
                rec2 = spool.tile([P, NBLK], DT, tag="rec2")
                nc.vector.tensor_scalar_add(rec2[:], pt2[:, :, 32], EPS)
                nc.vector.reciprocal(rec2[:], rec2[:])
                vst2 = spool.tile([P, NBLK, 32], DT, tag="vst2")
                nc.vector.tensor_tensor(out=vst2[:], in0=pt2[:, :, 0:32],
                                        in1=rec2[:].to_broadcast([P, NBLK, 32]),
                                        op=mybir.AluOpType.mult)
                hf = spool.tile([P, NBLK, 32], DT, tag="hf")
                for u in range(0, NBLK, 4):
                    nu = min(4, NBLK - u)
                    tp2 = pspool.tile([32 * nu, P], DT, space="PSUM", tag="tps")
                    nc.tensor.transpose(
                        out=tp2[:],
                        in_=vst2[:, u:u + nu, :].rearrange("p a b -> p (a b)"),
                        identity=ident[:])
                    t2s = zpool.tile([32 * nu, P], DT, tag="t2s")
                    nc.vector.tensor_copy(out=t2s[:], in_=tp2[:])
                    hp2 = pspool.tile([P, nu * 32], DT, space="PSUM", tag="hps")
                    nc.tensor.matmul(hp2[:], t2s[:], W2diag[0:32 * nu, 0:nu * 32],
                                     start=True, stop=True)
                    nc.scalar.activation(
                        hf[:, u:u + nu, :],
                        hp2[:].rearrange("p (a b) -> p a b", a=nu),
                        mybir.ActivationFunctionType.Relu)
                tmp3 = spool.tile([P, NBLK, 32], DT, tag="tmp3")
                nc.vector.tensor_tensor(out=tmp3[:], in0=hf[:],
                                        in1=Wlbc.rearrange("p (o w) -> p o w", o=1).to_broadcast([P, NBLK, 32]),
                                        op=mybir.AluOpType.mult)
                ycol = spool.tile([P, NBLK], DT, tag="ycol")
                nc.vector.tensor_reduce(out=ycol[:], in_=tmp3[:],
                                        axis=mybir.AxisListType.X,
                                        op=mybir.AluOpType.add)
                if bl != 0.0:
                    nc.vector.tensor_scalar_add(ycol[:], ycol[:], bl)
                nc.sync.dma_start(
                    y_d[:].rearrange("(g p) w -> p (g w)", p=P), ycol[:])
                f2pool.__exit__(None, None, None)

            for _ in range(replicate):
                body()

    nc.compile()
    return nc


def build_consts(weights):
    W1 = weights["W1"].astype(np.float32)
    W2 = weights["W2"].astype(np.float32)
    vs2 = (W2 @ weights["att_src2"]).astype(np.float32)
    vd2 = (W2 @ weights["att_dst2"]).astype(np.float32)
    Wl = weights["Wl"][:, 0].astype(np.float32)
    ct = np.zeros((P, C_END), np.float32)
    for u in range(4):
        ct[3 * u:3 * u + 3, C_W1 + 32 * u:C_W1 + 32 * u + 32] = W1
    for u in range(4):
        ct[32 * u:32 * u + 32, C_W2 + 32 * u:C_W2 + 32 * u + 32] = W2
    ct[:, C_VS2:C_VS2 + 32] = vs2[None, :]
    ct[:, C_VD2:C_VD2 + 32] = vd2[None, :]
    ct[:, C_WL:C_WL + 32] = Wl[None, :]
    ct[:, C_ID:C_ID + 128] = np.eye(P, dtype=np.float32)
    for w in range(3):
        ct[:, C_W1R + 32 * w:C_W1R + 32 * (w + 1)] = W1[w][None, :]
    # local pad mask [P, NBLK]: local row l = g*128+p, pad if l%CHK >= REAL
    l = np.arange(NPC)
    padl = (l % CHK) >= REAL
    ct[:, C_LM:C_LM + NBLK] = padl.reshape(NBLK, P).T * A_S_PAD
    # bidx pad mask [P, NC*NBLK]: bidx = G*128+p, pad if bidx%CHK >= REAL
    bi = np.arange(NRANK)
    padb = (bi % CHK) >= REAL
    ct[:, C_XM:C_XM + NC * NBLK] = padb.reshape(NC * NBLK, P).T * A_S_PAD
    return ct


def build_inputs(x, prep, weights):
    node2rank = prep["node2rank"]
    xr = np.zeros((NRANK, 3), np.float32)       # rank order
    xr[node2rank] = x
    bidx = rank_to_bidx(np.arange(NRANK))
    xrb = np.zeros((NRANK, 3), np.float32)      # bidx order
    xrb[bidx] = xr
    xfull = xrb.reshape(NC * NBLK, P, 3).transpose(1, 0, 2).copy()
    ct = build_consts(weights)
    per_core = []
    for c in range(NC):
        xl = xr[c * NPC:(c + 1) * NPC]  # [NPC, 3] local rank order
        xbp = np.zeros((P, NB, NBLK, 3), np.float32)
        for b in range(NB):
            xp = xl[prep["perms"][c, b]]
            xbp[:, b] = xp.reshape(NBLK, P, 3).transpose(1, 0, 2)
        per_core.append({
            "xfull": xfull, "xb": xbp,
            "gidx": prep["gidx"][c], "sidx": prep["sidx"][c],
            "adidx": prep["adidx"][c], "consts": ct,
        })
    return per_core


_CACHE = {}
LAST_EXEC_NS = None
LAST_RESULTS = None
LAST_NC = None
LAST_IN_MAPS = None


def kernel(**inputs):
    x = np.asarray(inputs["x"], np.float32)
    edge_index = np.asarray(inputs["edge_index"])
    weights = {k: np.asarray(v, np.float32) for k, v in inputs.items()
               if k not in ("x", "edge_index")}

    key = edge_index.tobytes()[:64]  # cheap cache key
    if key not in _CACHE:
        prep = preprocess(edge_index)
        nc = build_program(prep, weights)
        _CACHE[key] = (prep, nc)
    prep, nc = _CACHE[key]

    in_maps = build_inputs(x, prep, weights)
    res = run_bass_kernel_spmd(nc, in_maps, core_ids=list(range(NC)))
    global LAST_EXEC_NS, LAST_RESULTS, LAST_NC, LAST_IN_MAPS
    LAST_EXEC_NS = res.exec_time_ns
    LAST_RESULTS = res
    LAST_NC = nc
    LAST_IN_MAPS = in_maps
    y = np.zeros((N, 1), np.float32)
    yr = np.concatenate([res.results[c]["y"] for c in range(NC)], axis=0)
    y[:, 0] = yr[prep["node2rank"], 0]
    return y


if __name__ == "__main__":
    d = np.load("/root/problem/work/inputs.npz")
    inp = {k: d[k] for k in d.files}
    y = kernel(**inp)
    y_ref = np.load("/root/problem/work/y_ref.npy")
    rel = np.abs(y - y_ref).max() / np.abs(y_ref).max()
    print("rel err:", rel)
